# revision 22
# baseline (speedup 1.0000x reference)
"""Two-layer GAT (PyG GATConv math) on 8 Trainium2 NeuronCores via Bass/Tile.

Sharding: nodes split into 8 contiguous ranges of 12500 (graph partitioning per
the sharding hint); each core aggregates the in-edges of its own nodes.
Within a core, nodes are relabeled in descending in-degree order so 128-node
windows have near-uniform degree.

Per layer:
  node phase  - h = x @ W (PE matmuls) and attention scores es/ed (DVE),
                written as 256-B bf16 table rows [h(64) | es(8) | pad];
                AllGather replicates the 100352-row table to every core.
  edge phase  - degree-padded dense layout: window w = 128 dst nodes on
                partitions, slot column k = k-th in-edge. Edges are gathered
                in bulk with dma_gather (InstDMAGatherAnt): int16 indices
                limit one gather to 32768 table rows, so each window's slots
                are split into 4 per-quarter bands (quarter q = cores 2q,2q+1)
                and fetched by 4 gathers per multi-window batch. Pad slots
                point at a phantom row with es=-1e30 so exp weight is 0.
                Softmax (max-subtraction elided: scores are O(5)), weighted
                sum, bias/activation - all nodes-on-partitions DVE/ACT ops.
"""
import sys

sys.path.insert(0, "/opt/trn_rl_repo")

import numpy as np
import ml_dtypes

import concourse.bass as bass
import concourse.bacc as bacc
import concourse.tile as tile
from concourse import mybir
from concourse.bass import AP
from concourse.masks import make_identity

F32 = mybir.dt.float32
BF16 = mybir.dt.bfloat16
I16 = mybir.dt.int16
AX = mybir.AxisListType.X
OP = mybir.AluOpType
AF = mybir.ActivationFunctionType

N = 100_000
F_IN = 512
H1, FH1 = 8, 8
D1 = H1 * FH1          # 64
C = 64
NCORES = 8
NLR = N // NCORES      # 12500 real nodes per core
PW = 128
NWIN = (NLR + PW - 1) // PW   # 98
NL = NWIN * PW         # 12544 (44 phantom rows per core)
GT = NCORES * NL       # 100352 table rows
QR = 2 * NL            # 25088 rows per int16-addressable quarter
NQ = 4
RL = 128               # table row stride in bf16 elems (256 B)
NEG = -1.0e30
XB = 2                 # windows per x-load batch   (NWIN % XB == 0)
SB = 7                 # windows per table-staging batch (NWIN % SB == 0)
BUDGET = 208           # max padded slot columns per gather batch
GCH = 64               # slot columns (8192 idxs) per dma_gather chunk


# ---------------------------------------------------------------- host planning
def _plan(edge_index):
    src = np.concatenate([edge_index[0], np.arange(N)]).astype(np.int64)
    dst = np.concatenate([edge_index[1], np.arange(N)]).astype(np.int64)
    owner = dst // NLR

    # in-degree rank within each core -> window (128-node groups)
    rank_of = np.empty(N, dtype=np.int64)
    for c in range(NCORES):
        d_c = dst[owner == c] - c * NLR
        deg = np.bincount(d_c, minlength=NLR)
        order0 = np.argsort(-deg, kind="stable")
        r = np.empty(NLR, dtype=np.int64)
        r[order0] = np.arange(NLR)
        rank_of[c * NLR : (c + 1) * NLR] = r

    # greedy class (position mod 4 = gather quarter) assignment: balance each
    # dst's in-edge sources across the 4 classes; 32 slots per class per
    # window (21 in the last window so real nodes stay below the phantom pad)
    sorder = np.argsort(src, kind="stable")
    d_s = dst[sorder]
    outdeg = np.bincount(src, minlength=N)
    starts = np.concatenate([[0], np.cumsum(outdeg)])
    proc = np.argsort(-outdeg, kind="stable")
    capleft = np.full((NCORES, NWIN, NQ), 32, np.int32)
    capleft[:, NWIN - 1, :] = (NLR - (NWIN - 1) * PW) // NQ
    cls = np.empty(N, np.int8)
    cntT = np.zeros((N, NQ), np.int32)
    for s in proc:
        dsts = d_s[starts[s] : starts[s + 1]]
        c, w = s // NLR, rank_of[s] // PW
        cd = cntT[dsts]
        sc = (cd * cd + cd).sum(axis=0).astype(np.float64)
        sc[capleft[c, w] <= 0] = np.inf
        q = int(np.argmin(sc))
        cls[s] = q
        capleft[c, w, q] -= 1
        cntT[dsts, q] += 1

    # final position: class q of window w occupies slots w*128 + 4*j + q
    pos_of = np.empty(N, dtype=np.int64)
    gsel = []
    for c in range(NCORES):
        g = np.arange(c * NLR, (c + 1) * NLR)
        key = rank_of[g] // PW * NQ + cls[g]
        o2 = np.argsort(key, kind="stable")
        kcnt = np.bincount(key[o2], minlength=NWIN * NQ)
        kstart = np.concatenate([[0], np.cumsum(kcnt)])[:-1]
        j = np.arange(NLR) - kstart[key[o2]]
        pos = (key[o2] // NQ) * PW + NQ * j + key[o2] % NQ
        pos_of[g[o2]] = pos
        order = np.empty(NLR, dtype=np.int64)
        order[pos] = o2
        gsel.append(g[order])

    # per-core edge tuples in table coordinates
    core_edges = []
    for c in range(NCORES):
        m = owner == c
        s_c, d_c = src[m], dst[m]
        pos = pos_of[d_c]                         # dst slot position
        srow = (s_c // NLR) * NL + pos_of[s_c]
        q = cls[s_c].astype(np.int64)             # source quarter (= srow % 4)
        r = srow // NQ                            # interleaved within-quarter row
        core_edges.append((pos, q, r))

    # band widths K4[w][q] shared across cores
    K4 = np.zeros((NWIN, NQ), dtype=np.int64)
    cnts = []
    for c in range(NCORES):
        pos, q, r = core_edges[c]
        key = (pos // PW) * (NQ * PW) + q * PW + (pos % PW)
        cnt = np.bincount(key, minlength=NWIN * NQ * PW).reshape(NWIN, NQ, PW)
        cnts.append((key, cnt))
        K4 = np.maximum(K4, cnt.max(axis=2))
    K4 = np.maximum(K4, 1)

    # batches of consecutive windows, each with per-quarter uniform band widths
    batches = []
    w = 0
    while w < NWIN:
        k4b = K4[w].copy()
        nw = 1
        while w + nw < NWIN:
            cand = np.maximum(k4b, K4[w + nw])
            if (nw + 1) * int(cand.sum()) > BUDGET:
                break
            k4b = cand
            nw += 1
        batches.append((w, nw, tuple(int(v) for v in k4b)))
        w += nw

    # idx stream layout: per batch b, per quarter q, a [16, nw*K4b[q]*8] block
    g16 = []          # start col16 of each (b, q) section
    t16 = 0
    for (w0, nw, k4b) in batches:
        row = []
        for q in range(NQ):
            row.append(t16)
            t16 += nw * k4b[q] * 8    # n_idx/16 = nw*K4b*128/16
        g16.append(row)
    TW16 = t16

    # per-core idx matrices
    idx_streams = []
    for c in range(NCORES):
        pos, q, r = core_edges[c]
        key, cnt = cnts[c]
        ordd = np.argsort(key, kind="stable")
        key_s, r_s = key[ordd], r[ordd]
        ccnt = np.bincount(key_s, minlength=NWIN * NQ * PW)
        starts = np.concatenate([[0], np.cumsum(ccnt)])[:-1]
        k_of = np.arange(len(key_s)) - starts[key_s]
        w_s = key_s // (NQ * PW)
        q_s = (key_s // PW) % NQ
        p_s = key_s % PW

        # map window -> (batch, wl, K4b, col16 base of its quarter sections)
        wb = np.zeros(NWIN, dtype=np.int64)
        wl = np.zeros(NWIN, dtype=np.int64)
        for b, (w0, nw, k4b) in enumerate(batches):
            wb[w0:w0 + nw] = b
            wl[w0:w0 + nw] = np.arange(nw)
        k4b_arr = np.array([k4b for (_, _, k4b) in batches], dtype=np.int64)
        g16_arr = np.array(g16, dtype=np.int64)

        b_s = wb[w_s]
        j = (wl[w_s] * k4b_arr[b_s, q_s] + k_of) * PW + p_s
        flat16 = g16_arr[b_s, q_s] + j // 16
        prow = j % 16

        base16 = np.full((16, TW16), NLR // NQ, dtype=np.int16)
        base16[prow, flat16] = r_s.astype(np.int16)
        idx_streams.append(np.tile(base16, (8, 1)))

    return {"gsel": gsel, "K4": K4, "batches": batches,
            "g16": g16, "TW16": TW16, "idx": idx_streams}


def _apx(base: AP, off: int, dims) -> AP:
    """AP with base's partition dim and explicit free [step, count] dims."""
    return AP(base.tensor, base.offset + off, [list(base.ap[0])] + [list(d) for d in dims])


# ---------------------------------------------------------------- device build
def _build(batches, g16, TW16):
    nc = bacc.Bacc("TRN2", target_bir_lowering=False, debug=False, num_devices=NCORES)

    xT = nc.dram_tensor("xT", [F_IN, NL], F32, kind="ExternalInput")
    w1 = nc.dram_tensor("w1", [F_IN, D1], F32, kind="ExternalInput")
    w2 = nc.dram_tensor("w2", [D1, C], F32, kind="ExternalInput")
    cvec = nc.dram_tensor("cvec", [128, 6 * 64], F32, kind="ExternalInput")
    negd = nc.dram_tensor("negd", [NL - NLR, RL], BF16, kind="ExternalInput")
    idxd = nc.dram_tensor("idxd", [128, TW16], I16, kind="ExternalInput")
    outd = nc.dram_tensor("outv", [NL, C], F32, kind="ExternalOutput")

    t1b = nc.dram_tensor("t1b", [NL, RL], BF16)
    T1 = nc.dram_tensor("T1", [GT, RL], BF16, addr_space="Shared")
    t2b = nc.dram_tensor("t2b", [NL, RL], BF16)
    T2 = nc.dram_tensor("T2", [GT, RL], BF16, addr_space="Shared")

    MAXC = max(nw * sum(k4b) for (_, nw, k4b) in batches)   # <= BUDGET
    MAXW = max(nw for (_, nw, k4b) in batches)

    with tile.TileContext(nc) as tc:
        with (
            tc.tile_pool(name="consts", bufs=1) as cpool,
            tc.tile_pool(name="persist", bufs=1) as ppool,
            tc.tile_pool(name="xload", bufs=2) as xpool,
            tc.tile_pool(name="stg", bufs=2) as stgpool,
            tc.tile_pool(name="gpool", bufs=2) as gpool,
            tc.tile_pool(name="ipool", bufs=2) as ipool,
            tc.tile_pool(name="small", bufs=2) as spool,
            tc.tile_pool(name="psum", bufs=4, space="PSUM") as pspool,
        ):
            # ---- constants (packed)
            w1sb = cpool.tile([128, 4 * D1], F32)
            nc.sync.dma_start(out=w1sb[:].rearrange("p (cc d) -> p cc d", cc=4), in_=w1[:, :].rearrange("(cc p) d -> p cc d", p=128))
            w2sb = cpool.tile([128, C], F32)
            nc.sync.dma_start(out=w2sb[:D1, :], in_=w2[:, :])
            cv = cpool.tile([128, 6 * 64], F32)
            nc.sync.dma_start(out=cv[:], in_=cvec[:, :])
            asrs = cv[:, 0:64]
            adss = cv[:, 64:128]
            a2ss = cv[:, 128:192]
            a2ds = cv[:, 192:256]
            b1s = cv[:, 256:320]
            b2s = cv[:, 320:384]
            ident = cpool.tile([128, 128], F32)
            make_identity(nc, ident[:])

            # ---- persistent
            x2st = ppool.tile([128, NWIN * D1], F32)
            edt = ppool.tile([128, NWIN * H1 + NWIN], BF16)  # ed1 | ed2

            def node_phase(layer):
                tb, Tg = (t1b, T1) if layer == 1 else (t2b, T2)
                for sb in range(0, NWIN, SB):
                    stg = stgpool.tile([128, SB * RL], BF16, tag="stg")
                    nc.vector.memset(stg[:], 0.0)
                    for w in range(sb, sb + SB):
                        wl = w - sb
                        if layer == 1 and w % XB == 0:
                            xb = xpool.tile([128, 4 * XB * 128], F32, tag="xb")
                            nc.sync.dma_start(
                                out=xb[:].rearrange("p (cc n) -> p cc n", cc=4),
                                in_=xT[:, w * 128 : (w + XB) * 128].rearrange(
                                    "(cc p) n -> p cc n", p=128
                                ),
                            )
                        ph = pspool.tile([128, D1], F32, tag="ph")
                        if layer == 1:
                            nn = XB * 128
                            for cc in range(4):
                                nc.tensor.matmul(
                                    out=ph[:],
                                    lhsT=_apx(xb[:], cc * nn + (w % XB) * 128, [[1, 128]]),
                                    rhs=_apx(w1sb[:], cc * D1, [[1, D1]]),
                                    start=(cc == 0),
                                    stop=(cc == 3),
                                )
                        else:
                            pt = pspool.tile([64, 128], F32, tag="pt")
                            nc.tensor.transpose(
                                out=pt[:],
                                in_=_apx(x2st[:], w * D1, [[1, D1]]),
                                identity=ident[:],
                            )
                            x1t = spool.tile([64, 128], F32, tag="x1t")
                            nc.vector.tensor_copy(out=x1t[:], in_=pt[:])
                            nc.tensor.matmul(
                                out=ph[:], lhsT=x1t[:], rhs=w2sb[:D1, :],
                                start=True, stop=True,
                            )
                        # h row (bf16 cast) + scores
                        nc.vector.tensor_copy(
                            out=_apx(stg[:], wl * RL, [[1, D1]]), in_=ph[:])
                        a_s = asrs if layer == 1 else a2ss
                        a_d = adss if layer == 1 else a2ds
                        tmp = spool.tile([128, 2 * D1], F32, tag="tmp")
                        nc.vector.tensor_tensor(out=tmp[:, :D1], in0=ph[:], in1=a_s, op=OP.mult)
                        nc.vector.tensor_tensor(out=tmp[:, D1:], in0=ph[:], in1=a_d, op=OP.mult)
                        est = spool.tile([128, 16], F32, tag="est")
                        if layer == 1:
                            nc.vector.tensor_reduce(
                                out=est[:, 0:H1],
                                in_=_apx(tmp[:], 0, [[FH1, H1], [1, FH1]]),
                                axis=AX, op=OP.add)
                            nc.vector.tensor_reduce(
                                out=est[:, 8:16],
                                in_=_apx(tmp[:], D1, [[FH1, H1], [1, FH1]]),
                                axis=AX, op=OP.add)
                            nc.vector.tensor_copy(
                                out=_apx(stg[:], wl * RL + D1, [[1, H1]]),
                                in_=est[:, 0:H1])
                            nc.vector.tensor_copy(
                                out=_apx(edt[:], w * H1, [[1, H1]]),
                                in_=est[:, 8:16])
                        else:
                            nc.vector.tensor_reduce(
                                out=est[:, 0:1],
                                in_=_apx(tmp[:], 0, [[1, C]]),
                                axis=AX, op=OP.add)
                            nc.vector.tensor_reduce(
                                out=est[:, 1:2],
                                in_=_apx(tmp[:], D1, [[1, C]]),
                                axis=AX, op=OP.add)
                            nc.vector.tensor_copy(
                                out=_apx(stg[:], wl * RL + D1, [[1, 1]]),
                                in_=est[:, 0:1])
                            nc.vector.tensor_copy(
                                out=_apx(edt[:], NWIN * H1 + w, [[1, 1]]),
                                in_=est[:, 1:2])
                    nc.sync.dma_start(
                        out=tb[sb * 128 : (sb + SB) * 128, :].rearrange(
                            "(w p) r -> p w r", p=128
                        ),
                        in_=stg[:].rearrange("p (w r) -> p w r", w=SB),
                    )
                # phantom rows (the padding-slot target) -> giant negative es
                nc.sync.dma_start(out=tb[NLR:NL, :], in_=negd[:, :])
                nc.gpsimd.collective_compute(
                    "AllGather", OP.bypass,
                    replica_groups=[list(range(NCORES))],
                    ins=[tb[:, :]], outs=[Tg[:, :]],
                )

            def edge_phase(layer):
                Tg = T1 if layer == 1 else T2
                NH = H1 if layer == 1 else 1
                for b, (w0, nw, k4b) in enumerate(batches):
                    cols = nw * sum(k4b)
                    idxT = ipool.tile([128, BUDGET * 8], I16, tag="idx")
                    nc.sync.dma_start(
                        out=idxT[:, 0 : cols * 8],
                        in_=idxd[:, g16[b][0] : g16[b][0] + cols * 8],
                    )
                    G = gpool.tile([128, BUDGET * RL], BF16, tag="G")
                    secoff = []
                    so = 0
                    for q in range(NQ):
                        secoff.append(so)
                        so += nw * k4b[q]
                    # the gather ucode mishandles large in_ap base offsets:
                    # quarter q is the strided row view {4r+q} (elem_step,
                    # tiny base offset q*RL, int16 r < QR)
                    for q in range(NQ):
                        seccols = nw * k4b[q]
                        ch = 0
                        while ch < seccols:
                            cc = min(GCH, seccols - ch)
                            nq = cc * 128
                            i16a = (g16[b][q] - g16[b][0]) + ch * 8
                            nc.gpsimd.dma_gather(
                                out_ap=_apx(G[:], (secoff[q] + ch) * RL,
                                            [[RL, cc], [1, RL]]),
                                in_ap=AP(Tg[:, :].tensor, q * RL,
                                         [[NQ * RL, QR], [1, RL]]),
                                idxs_ap=idxT[:, i16a : i16a + nq // 16],
                                num_idxs=nq,
                                num_idxs_reg=nq,
                                elem_size=RL,
                                elem_step=NQ * RL,
                                single_packet=False,
                            )
                            ch += cc
                    # z = es + ed  (into the es slots, per quarter section)
                    for q in range(NQ):
                        if layer == 1:
                            nc.vector.tensor_tensor(
                                out=_apx(G[:], secoff[q] * RL + D1,
                                         [[k4b[q] * RL, nw], [RL, k4b[q]], [1, H1]]),
                                in0=_apx(G[:], secoff[q] * RL + D1,
                                         [[k4b[q] * RL, nw], [RL, k4b[q]], [1, H1]]),
                                in1=_apx(edt[:], w0 * H1,
                                         [[H1, nw], [0, k4b[q]], [1, H1]]),
                                op=OP.add)
                        else:
                            nc.vector.tensor_tensor(
                                out=_apx(G[:], secoff[q] * RL + D1,
                                         [[k4b[q] * RL, nw], [RL, k4b[q]]]),
                                in0=_apx(G[:], secoff[q] * RL + D1,
                                         [[k4b[q] * RL, nw], [RL, k4b[q]]]),
                                in1=_apx(edt[:], NWIN * H1 + w0,
                                         [[1, nw], [0, k4b[q]]]),
                                op=OP.add)
                    # leaky relu + exp over all es slots of the batch
                    zf = _apx(G[:], D1, [[RL, cols], [1, NH]])
                    nc.vector.scalar_tensor_tensor(
                        out=zf, in0=zf, scalar=0.2, in1=zf, op0=OP.mult, op1=OP.max)
                    nc.scalar.activation(out=zf, in_=zf, func=AF.Exp)
                    # denominators
                    den4 = spool.tile([128, 4 * MAXW * H1], F32, tag="den4")
                    for q in range(NQ):
                        nc.vector.tensor_reduce(
                            out=_apx(den4[:], q * nw * NH, [[1, nw * NH]]),
                            in_=_apx(G[:], secoff[q] * RL + D1,
                                     [[k4b[q] * RL, nw], [1, NH], [RL, k4b[q]]]),
                            axis=AX, op=OP.add)
                    den = spool.tile([128, MAXW * H1], F32, tag="den")
                    nc.vector.tensor_reduce(
                        out=_apx(den[:], 0, [[1, nw * NH]]),
                        in_=_apx(den4[:], 0, [[1, nw * NH], [nw * NH, 4]]),
                        axis=AX, op=OP.add)
                    if layer == 2:
                        # window-pad nodes have no edges: den 0 -> guard 0/0
                        nc.vector.tensor_scalar_add(
                            _apx(den[:], 0, [[1, nw]]),
                            _apx(den[:], 0, [[1, nw]]), 1e-30)
                    rden = spool.tile([128, MAXW * H1], F32, tag="rden")
                    nc.vector.reciprocal(
                        out=_apx(rden[:], 0, [[1, nw * NH]]),
                        in_=_apx(den[:], 0, [[1, nw * NH]]))
                    # weight the h entries by exp(z)
                    if layer == 1:
                        gh = _apx(G[:], 0, [[RL, cols], [FH1, H1], [1, FH1]])
                        nc.vector.tensor_tensor(
                            out=gh, in0=gh,
                            in1=_apx(G[:], D1, [[RL, cols], [1, H1], [0, FH1]]),
                            op=OP.mult)
                    else:
                        gh = _apx(G[:], 0, [[RL, cols], [1, C]])
                        nc.vector.tensor_tensor(
                            out=gh, in0=gh,
                            in1=_apx(G[:], D1, [[RL, cols], [0, C]]),
                            op=OP.mult)
                    # weighted sums
                    hs4 = spool.tile([128, 4 * MAXW * D1], F32, tag="hs4")
                    for q in range(NQ):
                        nc.vector.tensor_reduce(
                            out=_apx(hs4[:], q * nw * D1, [[1, nw * D1]]),
                            in_=_apx(G[:], secoff[q] * RL,
                                     [[k4b[q] * RL, nw], [1, D1], [RL, k4b[q]]]),
                            axis=AX, op=OP.add)
                    hsum = spool.tile([128, MAXW * D1], F32, tag="hsum")
                    nc.vector.tensor_reduce(
                        out=_apx(hsum[:], 0, [[1, nw * D1]]),
                        in_=_apx(hs4[:], 0, [[1, nw * D1], [nw * D1, 4]]),
                        axis=AX, op=OP.add)
                    if layer == 1:
                        nc.vector.tensor_tensor(
                            out=_apx(x2st[:], w0 * D1, [[1, nw * D1]]),
                            in0=_apx(hsum[:], 0, [[1, nw * D1]]),
                            in1=_apx(rden[:], 0, [[H1, nw], [1, H1], [0, FH1]]),
                            op=OP.mult)
                    else:
                        nc.vector.tensor_tensor(
                            out=_apx(x2st[:], w0 * C, [[1, nw * C]]),
                            in0=_apx(hsum[:], 0, [[1, nw * C]]),
                            in1=_apx(rden[:], 0, [[1, nw], [0, C]]),
                            op=OP.mult)

            # ================= layer 1 =================
            node_phase(1)
            edge_phase(1)
            # x1 = elu(x2st + b1), chunked
            for g in range(0, NWIN, SB):
                xs = _apx(x2st[:], g * D1, [[1, SB * D1]])
                nc.vector.tensor_tensor(
                    out=xs, in0=xs, in1=_apx(b1s, 0, [[0, SB], [1, D1]]), op=OP.add)
                tmp = spool.tile([128, SB * D1], F32, tag="tail")
                tf = _apx(tmp[:], 0, [[1, SB * D1]])
                nc.vector.tensor_scalar_min(tf, xs, 0.0)
                nc.scalar.activation(out=tf, in_=tf, func=AF.Exp)
                nc.vector.tensor_scalar_max(xs, xs, 0.0)
                nc.vector.scalar_tensor_tensor(
                    out=xs, in0=tf, scalar=-1.0, in1=xs, op0=OP.add, op1=OP.add)

            # ================= layer 2 =================
            node_phase(2)
            edge_phase(2)
            # out = log_softmax(x2st + b2), chunked
            for g in range(0, NWIN, SB):
                xs = _apx(x2st[:], g * C, [[1, SB * C]])
                nc.vector.tensor_tensor(
                    out=xs, in0=xs, in1=_apx(b2s, 0, [[0, SB], [1, C]]), op=OP.add)
                rmx = spool.tile([128, SB], F32, tag="rmx")
                nc.vector.tensor_reduce(
                    out=rmx[:], in_=_apx(x2st[:], g * C, [[C, SB], [1, C]]),
                    axis=AX, op=OP.max)
                nc.vector.tensor_tensor(
                    out=xs, in0=xs, in1=_apx(rmx[:], 0, [[1, SB], [0, C]]),
                    op=OP.subtract)
                tmp = spool.tile([128, SB * C], F32, tag="tail")
                tf = _apx(tmp[:], 0, [[1, SB * C]])
                nc.scalar.activation(out=tf, in_=xs, func=AF.Exp)
                nc.vector.tensor_reduce(
                    out=rmx[:], in_=_apx(tmp[:], 0, [[C, SB], [1, C]]),
                    axis=AX, op=OP.add)
                nc.scalar.activation(out=rmx[:], in_=rmx[:], func=AF.Ln)
                nc.vector.tensor_tensor(
                    out=xs, in0=xs, in1=_apx(rmx[:], 0, [[1, SB], [0, C]]),
                    op=OP.subtract)
            nc.sync.dma_start(
                out=outd[:, :].rearrange("(w p) f -> p w f", p=128),
                in_=x2st[:].rearrange("p (w f) -> p w f", w=NWIN),
            )

    nc.compile()
    return nc


# ---------------------------------------------------------------- PJRT runner
def _make_runner(nc):
    import jax
    from jax.sharding import Mesh, PartitionSpec, NamedSharding
    from jax.experimental.shard_map import shard_map
    from concourse import bass2jax
    from concourse.bass2jax import _bass_exec_p, install_neuronx_cc_hook

    install_neuronx_cc_hook()
    partition_name = nc.partition_id_tensor.name if nc.partition_id_tensor else None
    in_names, out_names, out_avals = [], [], []
    for alloc in nc.m.functions[0].allocations:
        if not isinstance(alloc, mybir.MemoryLocationSet):
            continue
        name = alloc.memorylocations[0].name
        if alloc.kind == "ExternalInput":
            if name != partition_name:
                in_names.append(name)
        elif alloc.kind == "ExternalOutput":
            out_avals.append(
                jax.core.ShapedArray(tuple(alloc.tensor_shape), mybir.dt.np(alloc.dtype))
            )
            out_names.append(name)
    n_params = len(in_names)
    all_in = list(in_names) + list(out_names)
    if partition_name is not None:
        all_in.append(partition_name)

    def _body(*args):
        operands = list(args)
        if partition_name is not None:
            operands.append(bass2jax.partition_id_tensor())
        return tuple(
            _bass_exec_p.bind(
                *operands,
                out_avals=tuple(out_avals),
                in_names=tuple(all_in),
                out_names=tuple(out_names),
                lowering_input_output_aliases=(),
                sim_require_finite=True,
                sim_require_nnan=True,
                nc=nc,
            )
        )

    devices = jax.devices()[:NCORES]
    mesh = Mesh(np.asarray(devices), ("core",))
    n_outs = len(out_names)
    sharded = jax.jit(
        shard_map(
            _body, mesh=mesh,
            in_specs=(PartitionSpec("core"),) * (n_params + n_outs),
            out_specs=(PartitionSpec("core"),) * n_outs,
            check_rep=False,
        ),
        keep_unused=True,
    )
    sharding = NamedSharding(mesh, PartitionSpec("core"))

    def run(in_maps):
        import jax as _jax

        per_core = [[np.asarray(m[nm]) for nm in in_names] for m in in_maps]
        concat_in = [
            np.concatenate([per_core[c][i] for c in range(NCORES)], axis=0)
            for i in range(n_params)
        ]
        concat_zero = [
            np.zeros((NCORES * a.shape[0], *a.shape[1:]), a.dtype) for a in out_avals
        ]
        args = [_jax.device_put(x, sharding) for x in concat_in + concat_zero]
        out = sharded(*args)
        _jax.block_until_ready(out)
        return (
            [
                {
                    nm: np.asarray(out[i]).reshape(NCORES, *out_avals[i].shape)[c]
                    for i, nm in enumerate(out_names)
                }
                for c in range(NCORES)
            ],
            sharded,
            args,
        )

    return run


_CACHE = {}


def _get_compiled(plan):
    key = (plan["TW16"], tuple(plan["batches"]))
    if key not in _CACHE:
        nc = _build(plan["batches"], plan["g16"], plan["TW16"])
        _CACHE[key] = (nc, _make_runner(nc))
    return _CACHE[key]


def _prep_inputs(x, plan, W1, att1_src, att1_dst, b1, W2, att2_src, att2_dst, b2):
    cvec = np.zeros((128, 6 * 64), np.float32)
    cvec[:, 0:64] = att1_src.reshape(1, D1)
    cvec[:, 64:128] = att1_dst.reshape(1, D1)
    cvec[:, 128:192] = att2_src.reshape(1, C)
    cvec[:, 192:256] = att2_dst.reshape(1, C)
    cvec[:, 256:320] = b1.reshape(1, D1)
    cvec[:, 320:384] = b2.reshape(1, C)
    in_maps = []
    for c in range(NCORES):
        xp = np.zeros((NL, F_IN), np.float32)
        xp[:NLR] = x[plan["gsel"][c]]
        in_maps.append(
            {
                "xT": np.ascontiguousarray(xp.T),
                "w1": np.ascontiguousarray(np.asarray(W1, np.float32)),
                "w2": np.ascontiguousarray(np.asarray(W2, np.float32)),
                "cvec": cvec,
                "negd": np.full((NL - NLR, RL), NEG, ml_dtypes.bfloat16),
                "idxd": plan["idx"][c],
            }
        )
    return in_maps


def kernel(x, edge_index, W1, att1_src, att1_dst, b1, W2, att2_src, att2_dst, b2):
    x = np.asarray(x, np.float32)
    edge_index = np.asarray(edge_index)
    plan = _plan(edge_index)
    nc, run = _get_compiled(plan)
    in_maps = _prep_inputs(
        x, plan,
        np.asarray(W1), np.asarray(att1_src), np.asarray(att1_dst), np.asarray(b1),
        np.asarray(W2), np.asarray(att2_src), np.asarray(att2_dst), np.asarray(b2),
    )
    results, _, _ = run(in_maps)
    out = np.empty((N, C), np.float32)
    for c in range(NCORES):
        out[plan["gsel"][c]] = results[c]["outv"][:NLR]
    return out


# revision 31
# speedup vs baseline: 1.6754x; 1.6754x over previous
"""Two-layer GAT (PyG GATConv math) on 8 Trainium2 NeuronCores via Bass/Tile.

Sharding: nodes split into 8 contiguous ranges of 12500 (graph partitioning per
the sharding hint); each core aggregates the in-edges of its own nodes.
Within a core, nodes are relabeled in descending in-degree order so 128-node
windows have near-uniform degree.

Per layer:
  node phase  - h = x @ W (PE matmuls) and attention scores es/ed (DVE),
                written as 256-B bf16 table rows [h(64) | es(8) | pad];
                AllGather replicates the 100352-row table to every core.
  edge phase  - degree-padded dense layout: window w = 128 dst nodes on
                partitions, slot column k = k-th in-edge. Edges are gathered
                in bulk with dma_gather (InstDMAGatherAnt): int16 indices
                limit one gather to 32768 table rows, so each window's slots
                are split into 4 per-quarter bands (quarter q = cores 2q,2q+1)
                and fetched by 4 gathers per multi-window batch. Pad slots
                point at a phantom row with es=-1e30 so exp weight is 0.
                Softmax (max-subtraction elided: scores are O(5)), weighted
                sum, bias/activation - all nodes-on-partitions DVE/ACT ops.
"""
import os
import sys

sys.path.insert(0, "/opt/trn_rl_repo")

import numpy as np
import ml_dtypes

ABL = os.environ.get("ABL", "")   # ablation switches for perf debugging

import concourse.bass as bass
import concourse.bacc as bacc
import concourse.tile as tile
from concourse import mybir
from concourse.bass import AP
from concourse.masks import make_identity

F32 = mybir.dt.float32
BF16 = mybir.dt.bfloat16
I16 = mybir.dt.int16
AX = mybir.AxisListType.X
OP = mybir.AluOpType
AF = mybir.ActivationFunctionType

N = 100_000
F_IN = 512
H1, FH1 = 8, 8
D1 = H1 * FH1          # 64
C = 64
NCORES = 8
NLR = N // NCORES      # 12500 real nodes per core
PW = 128
NWIN = (NLR + PW - 1) // PW   # 98
NL = NWIN * PW         # 12544 (44 phantom rows per core)
GT = NCORES * NL       # 100352 table rows
QR = 2 * NL            # 25088 rows per int16-addressable quarter
NQ = 4
RL = 128               # table row stride in bf16 elems (256 B)
NEG = -1.0e30
XB = 2                 # windows per x-load batch   (NWIN % XB == 0)
SB = 7                 # windows per table-staging batch (NWIN % SB == 0)
BUDGET = 208           # max padded slot columns per gather batch
GCH = 64               # slot columns (8192 idxs) per dma_gather chunk


# ---------------------------------------------------------------- host planning
def _plan(edge_index):
    src = np.concatenate([edge_index[0], np.arange(N)]).astype(np.int64)
    dst = np.concatenate([edge_index[1], np.arange(N)]).astype(np.int64)
    owner = dst // NLR

    # in-degree rank within each core -> window (128-node groups)
    rank_of = np.empty(N, dtype=np.int64)
    for c in range(NCORES):
        d_c = dst[owner == c] - c * NLR
        deg = np.bincount(d_c, minlength=NLR)
        order0 = np.argsort(-deg, kind="stable")
        r = np.empty(NLR, dtype=np.int64)
        r[order0] = np.arange(NLR)
        rank_of[c * NLR : (c + 1) * NLR] = r

    # greedy class (position mod 4 = gather quarter) assignment: balance each
    # dst's in-edge sources across the 4 classes; 32 slots per class per
    # window (21 in the last window so real nodes stay below the phantom pad)
    sorder = np.argsort(src, kind="stable")
    d_s = dst[sorder]
    outdeg = np.bincount(src, minlength=N)
    starts = np.concatenate([[0], np.cumsum(outdeg)])
    proc = np.argsort(-outdeg, kind="stable")
    capleft = np.full((NCORES, NWIN, NQ), 32, np.int32)
    capleft[:, NWIN - 1, :] = (NLR - (NWIN - 1) * PW) // NQ
    cls = np.empty(N, np.int8)
    cntT = np.zeros((N, NQ), np.int32)
    for s in proc:
        dsts = d_s[starts[s] : starts[s + 1]]
        c, w = s // NLR, rank_of[s] // PW
        cd = cntT[dsts]
        sc = (cd * cd + cd).sum(axis=0).astype(np.float64)
        sc[capleft[c, w] <= 0] = np.inf
        q = int(np.argmin(sc))
        cls[s] = q
        capleft[c, w, q] -= 1
        cntT[dsts, q] += 1

    # final position: class q of window w occupies slots w*128 + 4*j + q
    pos_of = np.empty(N, dtype=np.int64)
    gsel = []
    for c in range(NCORES):
        g = np.arange(c * NLR, (c + 1) * NLR)
        key = rank_of[g] // PW * NQ + cls[g]
        o2 = np.argsort(key, kind="stable")
        kcnt = np.bincount(key[o2], minlength=NWIN * NQ)
        kstart = np.concatenate([[0], np.cumsum(kcnt)])[:-1]
        j = np.arange(NLR) - kstart[key[o2]]
        pos = (key[o2] // NQ) * PW + NQ * j + key[o2] % NQ
        pos_of[g[o2]] = pos
        order = np.empty(NLR, dtype=np.int64)
        order[pos] = o2
        gsel.append(g[order])

    # per-core edge tuples in table coordinates
    core_edges = []
    for c in range(NCORES):
        m = owner == c
        s_c, d_c = src[m], dst[m]
        pos = pos_of[d_c]                         # dst slot position
        srow = (s_c // NLR) * NL + pos_of[s_c]
        q = cls[s_c].astype(np.int64)             # source quarter (= srow % 4)
        r = srow // NQ                            # interleaved within-quarter row
        core_edges.append((pos, q, r))

    # band widths K4[w][q] shared across cores
    K4 = np.zeros((NWIN, NQ), dtype=np.int64)
    cnts = []
    for c in range(NCORES):
        pos, q, r = core_edges[c]
        key = (pos // PW) * (NQ * PW) + q * PW + (pos % PW)
        cnt = np.bincount(key, minlength=NWIN * NQ * PW).reshape(NWIN, NQ, PW)
        cnts.append((key, cnt))
        K4 = np.maximum(K4, cnt.max(axis=2))
    K4 = np.maximum(K4, 1)

    # batches of consecutive windows, each with per-quarter uniform band widths
    batches = []
    w = 0
    while w < NWIN:
        k4b = K4[w].copy()
        nw = 1
        while w + nw < NWIN:
            cand = np.maximum(k4b, K4[w + nw])
            if (nw + 1) * int(cand.sum()) > BUDGET:
                break
            k4b = cand
            nw += 1
        batches.append((w, nw, tuple(int(v) for v in k4b)))
        w += nw

    # idx stream layout: per batch b, per quarter q, a [16, nw*K4b[q]*8] block
    g16 = []          # start col16 of each (b, q) section
    t16 = 0
    for (w0, nw, k4b) in batches:
        row = []
        for q in range(NQ):
            row.append(t16)
            t16 += nw * k4b[q] * 8    # n_idx/16 = nw*K4b*128/16
        g16.append(row)
    TW16 = t16

    # per-core idx matrices
    idx_streams = []
    for c in range(NCORES):
        pos, q, r = core_edges[c]
        key, cnt = cnts[c]
        ordd = np.argsort(key, kind="stable")
        key_s, r_s = key[ordd], r[ordd]
        ccnt = np.bincount(key_s, minlength=NWIN * NQ * PW)
        starts = np.concatenate([[0], np.cumsum(ccnt)])[:-1]
        k_of = np.arange(len(key_s)) - starts[key_s]
        w_s = key_s // (NQ * PW)
        q_s = (key_s // PW) % NQ
        p_s = key_s % PW

        # map window -> (batch, wl, K4b, col16 base of its quarter sections)
        wb = np.zeros(NWIN, dtype=np.int64)
        wl = np.zeros(NWIN, dtype=np.int64)
        for b, (w0, nw, k4b) in enumerate(batches):
            wb[w0:w0 + nw] = b
            wl[w0:w0 + nw] = np.arange(nw)
        k4b_arr = np.array([k4b for (_, _, k4b) in batches], dtype=np.int64)
        g16_arr = np.array(g16, dtype=np.int64)

        b_s = wb[w_s]
        j = (wl[w_s] * k4b_arr[b_s, q_s] + k_of) * PW + p_s
        flat16 = g16_arr[b_s, q_s] + j // 16
        prow = j % 16

        base16 = np.full((16, TW16), NLR // NQ, dtype=np.int16)
        base16[prow, flat16] = r_s.astype(np.int16)
        idx_streams.append(np.tile(base16, (8, 1)))

    return {"gsel": gsel, "K4": K4, "batches": batches,
            "g16": g16, "TW16": TW16, "idx": idx_streams}


def _apx(base: AP, off: int, dims) -> AP:
    """AP with base's partition dim and explicit free [step, count] dims."""
    return AP(base.tensor, base.offset + off, [list(base.ap[0])] + [list(d) for d in dims])


# ---------------------------------------------------------------- device build
def _build(batches, g16, TW16):
    nc = bacc.Bacc("TRN2", target_bir_lowering=False, debug=False, num_devices=NCORES,
                   num_swdge_queues=4)

    xT = nc.dram_tensor("xT", [F_IN, NL], F32, kind="ExternalInput")
    w1 = nc.dram_tensor("w1", [F_IN, D1], F32, kind="ExternalInput")
    w2 = nc.dram_tensor("w2", [D1, C], F32, kind="ExternalInput")
    cvec = nc.dram_tensor("cvec", [128, 6 * 64], F32, kind="ExternalInput")
    negd = nc.dram_tensor("negd", [NL - NLR, RL], BF16, kind="ExternalInput")
    idxd = nc.dram_tensor("idxd", [128, TW16], I16, kind="ExternalInput")
    outd = nc.dram_tensor("outv", [NL, C], F32, kind="ExternalOutput")

    t1b = nc.dram_tensor("t1b", [NL, RL], BF16)
    T1 = nc.dram_tensor("T1", [GT, RL], BF16, addr_space="Shared")
    t2b = nc.dram_tensor("t2b", [NL, RL], BF16)
    T2 = nc.dram_tensor("T2", [GT, RL], BF16, addr_space="Shared")

    MAXC = max(nw * sum(k4b) for (_, nw, k4b) in batches)   # <= BUDGET
    MAXW = max(nw for (_, nw, k4b) in batches)

    with tile.TileContext(nc) as tc:
        with (
            tc.tile_pool(name="consts", bufs=1) as cpool,
            tc.tile_pool(name="persist", bufs=1) as ppool,
            tc.tile_pool(name="xload", bufs=2) as xpool,
            tc.tile_pool(name="stg", bufs=2) as stgpool,
            tc.tile_pool(name="gpool", bufs=2) as gpool,
            tc.tile_pool(name="ipool", bufs=2) as ipool,
            tc.tile_pool(name="small", bufs=2) as spool,
            tc.tile_pool(name="psum", bufs=4, space="PSUM") as pspool,
        ):
            # ---- constants (packed)
            w1sb = cpool.tile([128, 4 * D1], F32)
            nc.sync.dma_start(out=w1sb[:].rearrange("p (cc d) -> p cc d", cc=4), in_=w1[:, :].rearrange("(cc p) d -> p cc d", p=128))
            w2sb = cpool.tile([128, C], F32)
            nc.sync.dma_start(out=w2sb[:D1, :], in_=w2[:, :])
            cv = cpool.tile([128, 6 * 64], F32)
            nc.sync.dma_start(out=cv[:], in_=cvec[:, :])
            asrs = cv[:, 0:64]
            adss = cv[:, 64:128]
            a2ss = cv[:, 128:192]
            a2ds = cv[:, 192:256]
            b1s = cv[:, 256:320]
            b2s = cv[:, 320:384]
            ident = cpool.tile([128, 128], F32)
            make_identity(nc, ident[:])

            # ---- persistent
            x2st = ppool.tile([128, NWIN * D1], F32)
            edt = ppool.tile([128, NWIN * H1 + NWIN], BF16)  # ed1 | ed2
            if ABL:
                nc.vector.memset(x2st[:], 0.01)

            def node_phase(layer):
                tb, Tg = (t1b, T1) if layer == 1 else (t2b, T2)
                for sb in range(0, NWIN, SB):
                    stg = stgpool.tile([128, SB * RL], BF16, tag="stg")
                    nc.vector.memset(stg[:], 0.0)
                    for w in range(sb, sb + SB):
                        wl = w - sb
                        if layer == 1 and w % XB == 0:
                            xb = xpool.tile([128, 4 * XB * 128], F32, tag="xb")
                            nc.sync.dma_start(
                                out=xb[:].rearrange("p (cc n) -> p cc n", cc=4),
                                in_=xT[:, w * 128 : (w + XB) * 128].rearrange(
                                    "(cc p) n -> p cc n", p=128
                                ),
                            )
                        ph = pspool.tile([128, D1], F32, tag="ph")
                        if layer == 1:
                            nn = XB * 128
                            for cc in range(4):
                                nc.tensor.matmul(
                                    out=ph[:],
                                    lhsT=_apx(xb[:], cc * nn + (w % XB) * 128, [[1, 128]]),
                                    rhs=_apx(w1sb[:], cc * D1, [[1, D1]]),
                                    start=(cc == 0),
                                    stop=(cc == 3),
                                )
                        else:
                            pt = pspool.tile([64, 128], F32, tag="pt")
                            nc.tensor.transpose(
                                out=pt[:],
                                in_=_apx(x2st[:], w * D1, [[1, D1]]),
                                identity=ident[:],
                            )
                            x1t = spool.tile([64, 128], F32, tag="x1t")
                            nc.vector.tensor_copy(out=x1t[:], in_=pt[:])
                            nc.tensor.matmul(
                                out=ph[:], lhsT=x1t[:], rhs=w2sb[:D1, :],
                                start=True, stop=True,
                            )
                        # h row (bf16 cast) + scores
                        nc.vector.tensor_copy(
                            out=_apx(stg[:], wl * RL, [[1, D1]]), in_=ph[:])
                        a_s = asrs if layer == 1 else a2ss
                        a_d = adss if layer == 1 else a2ds
                        tmp = spool.tile([128, 2 * D1], F32, tag="tmp")
                        nc.vector.tensor_tensor(out=tmp[:, :D1], in0=ph[:], in1=a_s, op=OP.mult)
                        nc.vector.tensor_tensor(out=tmp[:, D1:], in0=ph[:], in1=a_d, op=OP.mult)
                        est = spool.tile([128, 16], F32, tag="est")
                        if layer == 1:
                            nc.vector.tensor_reduce(
                                out=est[:, 0:H1],
                                in_=_apx(tmp[:], 0, [[FH1, H1], [1, FH1]]),
                                axis=AX, op=OP.add)
                            nc.vector.tensor_reduce(
                                out=est[:, 8:16],
                                in_=_apx(tmp[:], D1, [[FH1, H1], [1, FH1]]),
                                axis=AX, op=OP.add)
                            nc.vector.tensor_copy(
                                out=_apx(stg[:], wl * RL + D1, [[1, H1]]),
                                in_=est[:, 0:H1])
                            nc.vector.tensor_copy(
                                out=_apx(edt[:], w * H1, [[1, H1]]),
                                in_=est[:, 8:16])
                        else:
                            nc.vector.tensor_reduce(
                                out=est[:, 0:1],
                                in_=_apx(tmp[:], 0, [[1, C]]),
                                axis=AX, op=OP.add)
                            nc.vector.tensor_reduce(
                                out=est[:, 1:2],
                                in_=_apx(tmp[:], D1, [[1, C]]),
                                axis=AX, op=OP.add)
                            nc.vector.tensor_copy(
                                out=_apx(stg[:], wl * RL + D1, [[1, 1]]),
                                in_=est[:, 0:1])
                            nc.vector.tensor_copy(
                                out=_apx(edt[:], NWIN * H1 + w, [[1, 1]]),
                                in_=est[:, 1:2])
                    nc.sync.dma_start(
                        out=tb[sb * 128 : (sb + SB) * 128, :].rearrange(
                            "(w p) r -> p w r", p=128
                        ),
                        in_=stg[:].rearrange("p (w r) -> p w r", w=SB),
                    )
                # phantom rows (the padding-slot target) -> giant negative es
                nc.sync.dma_start(out=tb[NLR:NL, :], in_=negd[:, :])
                nc.gpsimd.collective_compute(
                    "AllGather", OP.bypass,
                    replica_groups=[list(range(NCORES))],
                    ins=[tb[:, :]], outs=[Tg[:, :]],
                )

            gq = [0]   # round-robin SWDGE queue for gather prep parallelism

            def edge_phase(layer):
                if "noedge" in ABL:
                    return
                Tg = T1 if layer == 1 else T2
                NH = H1 if layer == 1 else 1
                for b, (w0, nw, k4b) in enumerate(batches):
                    cols = nw * sum(k4b)
                    idxT = ipool.tile([128, BUDGET * 8], I16, tag="idx")
                    nc.sync.dma_start(
                        out=idxT[:, 0 : cols * 8],
                        in_=idxd[:, g16[b][0] : g16[b][0] + cols * 8],
                    )
                    G = gpool.tile([128, BUDGET * RL], BF16, tag="G")
                    secoff = []
                    so = 0
                    for q in range(NQ):
                        secoff.append(so)
                        so += nw * k4b[q]
                    # the gather ucode mishandles large in_ap base offsets:
                    # quarter q is the strided row view {4r+q} (elem_step,
                    # tiny base offset q*RL, int16 r < QR)
                    for q in range(NQ if "nogather" not in ABL else 0):
                        seccols = nw * k4b[q]
                        ch = 0
                        while ch < seccols:
                            cc = min(GCH, seccols - ch)
                            nq = cc * 128
                            i16a = (g16[b][q] - g16[b][0]) + ch * 8
                            nc.gpsimd.dma_gather(
                                out_ap=_apx(G[:], (secoff[q] + ch) * RL,
                                            [[RL, cc], [1, RL]]),
                                in_ap=AP(Tg[:, :].tensor, q * RL,
                                         [[NQ * RL, QR], [1, RL]]),
                                idxs_ap=idxT[:, i16a : i16a + nq // 16],
                                num_idxs=nq,
                                num_idxs_reg=nq,
                                elem_size=RL,
                                elem_step=NQ * RL,
                                single_packet=False,
                                queue_num=gq[0] % 4,
                            )
                            gq[0] += 1
                            ch += cc
                    if "nocompute" in ABL:
                        continue
                    # z = es + ed  (into the es slots, per quarter section)
                    for q in range(NQ):
                        if layer == 1:
                            nc.vector.tensor_tensor(
                                out=_apx(G[:], secoff[q] * RL + D1,
                                         [[k4b[q] * RL, nw], [RL, k4b[q]], [1, H1]]),
                                in0=_apx(G[:], secoff[q] * RL + D1,
                                         [[k4b[q] * RL, nw], [RL, k4b[q]], [1, H1]]),
                                in1=_apx(edt[:], w0 * H1,
                                         [[H1, nw], [0, k4b[q]], [1, H1]]),
                                op=OP.add)
                        else:
                            nc.vector.tensor_tensor(
                                out=_apx(G[:], secoff[q] * RL + D1,
                                         [[k4b[q] * RL, nw], [RL, k4b[q]]]),
                                in0=_apx(G[:], secoff[q] * RL + D1,
                                         [[k4b[q] * RL, nw], [RL, k4b[q]]]),
                                in1=_apx(edt[:], NWIN * H1 + w0,
                                         [[1, nw], [0, k4b[q]]]),
                                op=OP.add)
                    # leaky relu + exp over all es slots of the batch
                    zf = _apx(G[:], D1, [[RL, cols], [1, NH]])
                    nc.vector.scalar_tensor_tensor(
                        out=zf, in0=zf, scalar=0.2, in1=zf, op0=OP.mult, op1=OP.max)
                    nc.scalar.activation(out=zf, in_=zf, func=AF.Exp)
                    # denominators
                    den4 = spool.tile([128, 4 * MAXW * H1], F32, tag="den4")
                    for q in range(NQ):
                        nc.vector.tensor_reduce(
                            out=_apx(den4[:], q * nw * NH, [[1, nw * NH]]),
                            in_=_apx(G[:], secoff[q] * RL + D1,
                                     [[k4b[q] * RL, nw], [1, NH], [RL, k4b[q]]]),
                            axis=AX, op=OP.add)
                    den = spool.tile([128, MAXW * H1], F32, tag="den")
                    nc.vector.tensor_reduce(
                        out=_apx(den[:], 0, [[1, nw * NH]]),
                        in_=_apx(den4[:], 0, [[1, nw * NH], [nw * NH, 4]]),
                        axis=AX, op=OP.add)
                    if layer == 2:
                        # window-pad nodes have no edges: den 0 -> guard 0/0
                        nc.vector.tensor_scalar_add(
                            _apx(den[:], 0, [[1, nw]]),
                            _apx(den[:], 0, [[1, nw]]), 1e-30)
                    rden = spool.tile([128, MAXW * H1], F32, tag="rden")
                    nc.vector.reciprocal(
                        out=_apx(rden[:], 0, [[1, nw * NH]]),
                        in_=_apx(den[:], 0, [[1, nw * NH]]))
                    # weight the h entries by exp(z)
                    if layer == 1:
                        gh = _apx(G[:], 0, [[RL, cols], [FH1, H1], [1, FH1]])
                        nc.vector.tensor_tensor(
                            out=gh, in0=gh,
                            in1=_apx(G[:], D1, [[RL, cols], [1, H1], [0, FH1]]),
                            op=OP.mult)
                    else:
                        gh = _apx(G[:], 0, [[RL, cols], [1, C]])
                        nc.vector.tensor_tensor(
                            out=gh, in0=gh,
                            in1=_apx(G[:], D1, [[RL, cols], [0, C]]),
                            op=OP.mult)
                    # weighted sums
                    hs4 = spool.tile([128, 4 * MAXW * D1], F32, tag="hs4")
                    for q in range(NQ):
                        nc.vector.tensor_reduce(
                            out=_apx(hs4[:], q * nw * D1, [[1, nw * D1]]),
                            in_=_apx(G[:], secoff[q] * RL,
                                     [[k4b[q] * RL, nw], [1, D1], [RL, k4b[q]]]),
                            axis=AX, op=OP.add)
                    hsum = spool.tile([128, MAXW * D1], F32, tag="hsum")
                    nc.vector.tensor_reduce(
                        out=_apx(hsum[:], 0, [[1, nw * D1]]),
                        in_=_apx(hs4[:], 0, [[1, nw * D1], [nw * D1, 4]]),
                        axis=AX, op=OP.add)
                    if layer == 1:
                        nc.vector.tensor_tensor(
                            out=_apx(x2st[:], w0 * D1, [[1, nw * D1]]),
                            in0=_apx(hsum[:], 0, [[1, nw * D1]]),
                            in1=_apx(rden[:], 0, [[H1, nw], [1, H1], [0, FH1]]),
                            op=OP.mult)
                    else:
                        nc.vector.tensor_tensor(
                            out=_apx(x2st[:], w0 * C, [[1, nw * C]]),
                            in0=_apx(hsum[:], 0, [[1, nw * C]]),
                            in1=_apx(rden[:], 0, [[1, nw], [0, C]]),
                            op=OP.mult)

            # ================= layer 1 =================
            node_phase(1)
            edge_phase(1)
            # x1 = elu(x2st + b1), chunked
            for g in range(0, NWIN, SB):
                xs = _apx(x2st[:], g * D1, [[1, SB * D1]])
                nc.vector.tensor_tensor(
                    out=xs, in0=xs, in1=_apx(b1s, 0, [[0, SB], [1, D1]]), op=OP.add)
                tmp = spool.tile([128, SB * D1], F32, tag="tail")
                tf = _apx(tmp[:], 0, [[1, SB * D1]])
                nc.vector.tensor_scalar_min(tf, xs, 0.0)
                nc.scalar.activation(out=tf, in_=tf, func=AF.Exp)
                nc.vector.tensor_scalar_max(xs, xs, 0.0)
                nc.vector.scalar_tensor_tensor(
                    out=xs, in0=tf, scalar=-1.0, in1=xs, op0=OP.add, op1=OP.add)

            # ================= layer 2 =================
            node_phase(2)
            edge_phase(2)
            # out = log_softmax(x2st + b2), chunked
            for g in range(0, NWIN, SB):
                xs = _apx(x2st[:], g * C, [[1, SB * C]])
                nc.vector.tensor_tensor(
                    out=xs, in0=xs, in1=_apx(b2s, 0, [[0, SB], [1, C]]), op=OP.add)
                rmx = spool.tile([128, SB], F32, tag="rmx")
                nc.vector.tensor_reduce(
                    out=rmx[:], in_=_apx(x2st[:], g * C, [[C, SB], [1, C]]),
                    axis=AX, op=OP.max)
                nc.vector.tensor_tensor(
                    out=xs, in0=xs, in1=_apx(rmx[:], 0, [[1, SB], [0, C]]),
                    op=OP.subtract)
                tmp = spool.tile([128, SB * C], F32, tag="tail")
                tf = _apx(tmp[:], 0, [[1, SB * C]])
                nc.scalar.activation(out=tf, in_=xs, func=AF.Exp)
                nc.vector.tensor_reduce(
                    out=rmx[:], in_=_apx(tmp[:], 0, [[C, SB], [1, C]]),
                    axis=AX, op=OP.add)
                nc.scalar.activation(out=rmx[:], in_=rmx[:], func=AF.Ln)
                nc.vector.tensor_tensor(
                    out=xs, in0=xs, in1=_apx(rmx[:], 0, [[1, SB], [0, C]]),
                    op=OP.subtract)
            nc.sync.dma_start(
                out=outd[:, :].rearrange("(w p) f -> p w f", p=128),
                in_=x2st[:].rearrange("p (w f) -> p w f", w=NWIN),
            )

    nc.compile()
    return nc


# ---------------------------------------------------------------- PJRT runner
def _make_runner(nc):
    import jax
    from jax.sharding import Mesh, PartitionSpec, NamedSharding
    from jax.experimental.shard_map import shard_map
    from concourse import bass2jax
    from concourse.bass2jax import _bass_exec_p, install_neuronx_cc_hook

    install_neuronx_cc_hook()
    partition_name = nc.partition_id_tensor.name if nc.partition_id_tensor else None
    in_names, out_names, out_avals = [], [], []
    for alloc in nc.m.functions[0].allocations:
        if not isinstance(alloc, mybir.MemoryLocationSet):
            continue
        name = alloc.memorylocations[0].name
        if alloc.kind == "ExternalInput":
            if name != partition_name:
                in_names.append(name)
        elif alloc.kind == "ExternalOutput":
            out_avals.append(
                jax.core.ShapedArray(tuple(alloc.tensor_shape), mybir.dt.np(alloc.dtype))
            )
            out_names.append(name)
    n_params = len(in_names)
    all_in = list(in_names) + list(out_names)
    if partition_name is not None:
        all_in.append(partition_name)

    def _body(*args):
        operands = list(args)
        if partition_name is not None:
            operands.append(bass2jax.partition_id_tensor())
        return tuple(
            _bass_exec_p.bind(
                *operands,
                out_avals=tuple(out_avals),
                in_names=tuple(all_in),
                out_names=tuple(out_names),
                lowering_input_output_aliases=(),
                sim_require_finite=True,
                sim_require_nnan=True,
                nc=nc,
            )
        )

    devices = jax.devices()[:NCORES]
    mesh = Mesh(np.asarray(devices), ("core",))
    n_outs = len(out_names)
    sharded = jax.jit(
        shard_map(
            _body, mesh=mesh,
            in_specs=(PartitionSpec("core"),) * (n_params + n_outs),
            out_specs=(PartitionSpec("core"),) * n_outs,
            check_rep=False,
        ),
        keep_unused=True,
    )
    sharding = NamedSharding(mesh, PartitionSpec("core"))

    def run(in_maps):
        import jax as _jax

        per_core = [[np.asarray(m[nm]) for nm in in_names] for m in in_maps]
        concat_in = [
            np.concatenate([per_core[c][i] for c in range(NCORES)], axis=0)
            for i in range(n_params)
        ]
        concat_zero = [
            np.zeros((NCORES * a.shape[0], *a.shape[1:]), a.dtype) for a in out_avals
        ]
        args = [_jax.device_put(x, sharding) for x in concat_in + concat_zero]
        out = sharded(*args)
        _jax.block_until_ready(out)
        return (
            [
                {
                    nm: np.asarray(out[i]).reshape(NCORES, *out_avals[i].shape)[c]
                    for i, nm in enumerate(out_names)
                }
                for c in range(NCORES)
            ],
            sharded,
            args,
        )

    return run


_CACHE = {}


def _get_compiled(plan):
    key = (ABL, plan["TW16"], tuple(plan["batches"]))
    if key not in _CACHE:
        nc = _build(plan["batches"], plan["g16"], plan["TW16"])
        _CACHE[key] = (nc, _make_runner(nc))
    return _CACHE[key]


def _prep_inputs(x, plan, W1, att1_src, att1_dst, b1, W2, att2_src, att2_dst, b2):
    cvec = np.zeros((128, 6 * 64), np.float32)
    cvec[:, 0:64] = att1_src.reshape(1, D1)
    cvec[:, 64:128] = att1_dst.reshape(1, D1)
    cvec[:, 128:192] = att2_src.reshape(1, C)
    cvec[:, 192:256] = att2_dst.reshape(1, C)
    cvec[:, 256:320] = b1.reshape(1, D1)
    cvec[:, 320:384] = b2.reshape(1, C)
    in_maps = []
    for c in range(NCORES):
        xp = np.zeros((NL, F_IN), np.float32)
        xp[:NLR] = x[plan["gsel"][c]]
        in_maps.append(
            {
                "xT": np.ascontiguousarray(xp.T),
                "w1": np.ascontiguousarray(np.asarray(W1, np.float32)),
                "w2": np.ascontiguousarray(np.asarray(W2, np.float32)),
                "cvec": cvec,
                "negd": np.full((NL - NLR, RL), NEG, ml_dtypes.bfloat16),
                "idxd": plan["idx"][c],
            }
        )
    return in_maps


def kernel(x, edge_index, W1, att1_src, att1_dst, b1, W2, att2_src, att2_dst, b2):
    x = np.asarray(x, np.float32)
    edge_index = np.asarray(edge_index)
    plan = _plan(edge_index)
    nc, run = _get_compiled(plan)
    in_maps = _prep_inputs(
        x, plan,
        np.asarray(W1), np.asarray(att1_src), np.asarray(att1_dst), np.asarray(b1),
        np.asarray(W2), np.asarray(att2_src), np.asarray(att2_dst), np.asarray(b2),
    )
    results, _, _ = run(in_maps)
    out = np.empty((N, C), np.float32)
    for c in range(NCORES):
        out[plan["gsel"][c]] = results[c]["outv"][:NLR]
    return out
